# revision 8
# baseline (speedup 1.0000x reference)
"""Chamfer distance loss on 8 Trainium2 cores.

Strategy (hardcoded for B=16, N=M=4096, D=3 fp32 inputs):
  - Data-parallel over batch: core c handles batches {2c, 2c+1}; each core
    returns a partial scalar sum; host adds the 8 partials and divides by B.
  - Per batch, the (4096 x 4096) squared-distance matrix is produced on the
    tensor engine as an augmented matmul dist = A^T @ Bm with K=16:
    each fp32 operand is split into (hi, lo) bf16 pairs so every product
    is exact in fp32 accumulation (bf16 matmul runs at full PE rate while
    true-fp32 matmul runs at 1/4 rate). Representation error ~2^-18.
  - Both reductions (min over axis 2 and min over axis 1) are computed as
    free-axis reductions by materializing the matrix in both orientations
    (swap stationary/moving operands).
  - The min-reduction is DVE-bound (only VectorE has min ops and its PSUM
    read port is 1 elem/cycle/lane). tensor_tensor_reduce consumes TWO
    tiles per cycle position (one from PSUM, one ACT-copied into SBUF),
    computing out = min(in0, in1) elementwise and accum = min-reduce(out)
    chained through an initial-value AP -> row mins with each distance
    element read exactly once at 2 elems/cycle aggregate.
"""

import sys

if "/opt/trn_rl_repo" not in sys.path:
    sys.path.insert(0, "/opt/trn_rl_repo")

import numpy as np
import ml_dtypes

BF16 = ml_dtypes.bfloat16

B, N, D = 16, 4096, 3
NCORES = 8
BPC = B // NCORES          # batches per core
KAUG = 16                  # augmented contraction dim (hi/lo split)
PT = 128                   # stationary points per matmul (psum partitions)
FT = 512                   # moving points per matmul (one psum bank fp32)
CH = 1024                  # chunk width per TTR operand (2 psum banks)
NI = N // PT               # 32 stationary tiles
GR = BPC * 2 * 2           # 8 groups of [KAUG, N]: (batch, orient, side)

_PROG = None


def _build_program(repeat=1):
    from concourse import bass, bacc, tile, mybir

    f32 = mybir.dt.float32
    bf = mybir.dt.bfloat16

    nc = bacc.Bacc("TRN2", target_bir_lowering=False, debug=False)
    ab_d = nc.declare_dram_parameter("ab", [KAUG, GR, N], bf, isOutput=False)
    out_d = nc.declare_dram_parameter("out", [1, 1], f32, isOutput=True)

    NCOL = BPC * 2 * NI  # 128 (batch, orient, i-tile) combos

    with tile.TileContext(nc) as tc:
        with (
            tc.tile_pool(name="io", bufs=1) as io_pool,
            tc.tile_pool(name="ps", bufs=2, space=bass.MemorySpace.PSUM) as ps_pool,
            tc.tile_pool(name="misc", bufs=1) as misc_pool,
        ):
            abt = io_pool.tile([KAUG, GR, N], bf)
            # per-group DMAs so compute starts once the first pair lands
            for g in range(GR):
                nc.sync.dma_start(out=abt[:, g, :], in_=ab_d[:, g, :])

            # per-(b,o,i) half-row mins: acc[:, half, col]
            acc = misc_pool.tile([PT, 2, NCOL], f32, tag="acc")

            for rep in range(repeat):
              for b in range(BPC):
                for o in range(2):
                    g_l = (b * 2 + o) * 2 + 0   # stationary side
                    g_r = (b * 2 + o) * 2 + 1   # moving side
                    for i in range(NI):
                        lhsT = abt[:, g_l, i * PT:(i + 1) * PT]
                        col = (b * 2 + o) * NI + i
                        for half in range(2):
                            base = half * 4 * FT
                            ps = ps_pool.tile([PT, 4 * FT], f32, tag="ps")
                            for q in range(4):
                                nc.tensor.matmul(
                                    ps[:, q * FT:(q + 1) * FT],
                                    lhsT,
                                    abt[:, g_r, base + q * FT: base + (q + 1) * FT],
                                    start=True, stop=True,
                                )
                            nc.vector.tensor_reduce(
                                out=acc[:, half, col:col + 1],
                                in_=ps[:],
                                axis=mybir.AxisListType.X,
                                op=mybir.AluOpType.min,
                            )

            # epilogue: rowmin = min(half0, half1); total = sum over all
            rm = misc_pool.tile([PT, NCOL], f32, tag="rm")
            nc.vector.tensor_tensor(
                out=rm[:], in0=acc[:, 0, :], in1=acc[:, 1, :],
                op=mybir.AluOpType.min,
            )
            rsum = misc_pool.tile([PT, 1], f32, tag="rsum")
            nc.vector.tensor_reduce(
                out=rsum[:], in_=rm[:],
                axis=mybir.AxisListType.X, op=mybir.AluOpType.add,
            )
            ones = misc_pool.tile([PT, 1], f32, tag="ones")
            nc.vector.memset(ones[:], 1.0)
            psc = ps_pool.tile([1, 1], f32, tag="ps")
            nc.tensor.matmul(psc[:], rsum[:], ones[:], start=True, stop=True)
            res = misc_pool.tile([1, 1], f32, tag="res")
            nc.vector.tensor_copy(res[:], psc[:])
            nc.sync.dma_start(out=out_d[:], in_=res[:])

    nc.compile()
    return nc


_PROGS = {}


def get_program(repeat=1):
    if repeat not in _PROGS:
        _PROGS[repeat] = _build_program(repeat)
    return _PROGS[repeat]


def _hi_lo(x):
    hi = x.astype(BF16)
    lo = (x - hi.astype(np.float32)).astype(BF16)
    return hi, lo


def _sides(a_pts, b_pts):
    """a_pts: stationary [n,3] fp32 (unscaled); b_pts: moving [m,3] fp32.

    Returns (A [KAUG,n], Bm [KAUG,m]) bf16 with A^T @ Bm == pairwise
    squared distances (exact in the hi/lo pair representation)."""
    n, m = len(a_pts), len(b_pts)
    sqa = np.sum(a_pts * a_pts, axis=-1, dtype=np.float32)
    sqb = np.sum(b_pts * b_pts, axis=-1, dtype=np.float32)
    bm = (-2.0 * b_pts).astype(np.float32)
    A = np.zeros((KAUG, n), BF16)
    Bm = np.zeros((KAUG, m), BF16)
    for d in range(D):
        ahi, alo = _hi_lo(a_pts[:, d])
        bhi, blo = _hi_lo(bm[:, d])
        A[4 * d + 0] = ahi
        A[4 * d + 1] = ahi
        A[4 * d + 2] = alo
        A[4 * d + 3] = alo
        Bm[4 * d + 0] = bhi
        Bm[4 * d + 1] = blo
        Bm[4 * d + 2] = bhi
        Bm[4 * d + 3] = blo
    shi, slo = _hi_lo(sqa)
    A[12] = shi
    A[13] = slo
    Bm[12] = 1
    Bm[13] = 1
    shi, slo = _hi_lo(sqb)
    A[14] = 1
    A[15] = 1
    Bm[14] = shi
    Bm[15] = slo
    return A, Bm


def build_inputs(p1, p2):
    """Per-core device input tensors: [NCORES][KAUG, GR, N] bf16."""
    ab = np.zeros((NCORES, KAUG, GR, N), BF16)
    for c in range(NCORES):
        for b in range(BPC):
            gb = c * BPC + b
            A1, B1 = _sides(p1[gb], p2[gb])
            A2, B2 = _sides(p2[gb], p1[gb])
            g = (b * 2 + 0) * 2
            ab[c, :, g + 0] = A1
            ab[c, :, g + 1] = B1
            g = (b * 2 + 1) * 2
            ab[c, :, g + 0] = A2
            ab[c, :, g + 1] = B2
    return ab


def run_cores(ab, trace=False, repeat=1):
    """Run the SPMD program over 8 cores; returns (partials [NCORES], results)."""
    from concourse.bass_utils import run_bass_kernel_spmd

    nc = get_program(repeat)
    in_maps = [{"ab": np.ascontiguousarray(ab[c])} for c in range(NCORES)]
    res = run_bass_kernel_spmd(nc, in_maps, list(range(NCORES)), trace=trace)
    partials = np.array(
        [np.float64(res.results[c]["out"][0, 0]) for c in range(NCORES)]
    )
    return partials, res


def kernel(points1, points2):
    p1 = np.asarray(points1, dtype=np.float32)
    p2 = np.asarray(points2, dtype=np.float32)
    ab = build_inputs(p1, p2)
    partials, _ = run_cores(ab, trace=False)
    return np.array(partials.sum() / B, dtype=np.float32)
